# revision 2
# baseline (speedup 1.0000x reference)
"""Trainium2 Bass kernel for the CLIP text/image concat multi-head classifier.

Full (unsharded) inputs in, full outputs out. 312 heads sharded 39-per-core
across 8 NeuronCores (head parallel); outputs concatenated along the class
axis on the host. No collectives.

v2 design (vs. baseline):
  - Text-side dot products (z_text = sum_d W_text[n,h,d]*text[n,d], the lin
    text terms, text norms) are B-independent per-class constants; they are
    folded into per-row biases on the host. This removes 44% of the
    steady-state weight DMA (the text half of C*_W1) and all N=1 matmuls.
  - 128-row tiles (rows padded 12168->12288, 96 tiles): full PE partition
    utilization (baseline used 104-row tiles = 81%).
  - Hidden pass per tile: 4|6 fp16 matmuls (ap 256) -> one ACT relu+bias
    into a persistent fp16 r buffer -> one paired bn_stats per 2 tiles.
  - BN tail (even/odd stat merge, var, rsqrt) batched over all 96 tiles in
    ~8 DVE/ACT ops on [128,96] arrays.
  - Output projection transposed: out[b,n] += r_tile[128rows,128b].T @
    At[128rows,39] accumulated over tiles, where At = (W2*gamma masked by
    head) * rsqrt(var+eps). Mean correction via a 257th r column and a
    final ones-row f32 matmul into the same PSUM group. ap=39 per matmul
    instead of the baseline's 257.
  - Outputs come out [B, 39] per core; host concat on axis 1, no transpose.
"""

import os
import sys
from contextlib import ExitStack

for _p in ("/opt/trn_rl_repo", "/root/.axon_site/_ro/trn_rl_repo"):
    if os.path.isdir(_p) and _p not in sys.path:
        sys.path.insert(0, _p)

import numpy as np
import concourse.bass as bass
import concourse.tile as tile
from concourse import bacc, mybir
from concourse.bass_utils import run_bass_kernel_spmd

F32 = mybir.dt.float32
F16 = mybir.dt.float16
AF = mybir.ActivationFunctionType
ALU = mybir.AluOpType

B, N, DE, DV, H = 256, 312, 512, 768, 312
EPS = 1e-5
NC = 8
NH = N // NC              # 39 heads per core
ROWS = NH * H             # 12168 (head, hidden) rows per core
TR = 128                  # rows per tile
NT = (ROWS + TR - 1) // TR  # 96 tiles (rows padded to 12288)
RP = NT * TR              # 12288 padded rows
C1D = DE // 128           # 4 contraction chunks
C2D = DV // 128           # 6
RW = B + 1                # r tile width: 256 batch + 1 mean col
CONCAT_AXIS = 1


class Ctx:
    pass


def _load_persistents(nc, tc, ctx, ins):
    c = Ctx()
    const = ctx.enter_context(tc.tile_pool(name="const", bufs=1))
    c.sp = ctx.enter_context(tc.tile_pool(name="sp", bufs=3))

    def ld(name, shape, dt):
        t = const.tile(shape, dt, tag=name)
        nc.sync.dma_start(t[:], ins[name][:])
        return t

    # order matters: imgT/ioutT first so PE can start early
    c.imgT = ld("imgT", [128, C1D, B], F16)
    c.ioutT = ld("ioutT", [128, C2D, B], F16)
    c.w1iT = ld("w1iT", [128, C1D, NH], F16)
    c.w2iT = ld("w2iT", [128, C2D, NH], F16)
    c.ttsT = ld("ttsT", [128, C1D, NH], F16)
    c.lb1r = ld("lb1r", [1, NH], F32)
    c.lb2r = ld("lb2r", [1, NH], F32)
    c.cst1r = ld("cst1r", [1, NH], F32)
    c.cst2r = ld("cst2r", [1, NH], F32)
    c.zb1 = ld("zb1", [128, NT], F32)
    c.zb2 = ld("zb2", [128, NT], F32)
    c.wg1 = ld("wg1", [128, NT, NH], F16)
    c.wg2 = ld("wg2", [128, NT, NH], F16)

    c.ones_col = const.tile([128, 1], F16, tag="ones_col")
    nc.vector.memset(c.ones_col[:], 1.0)
    c.ones_rowf = const.tile([1, 128], F32, tag="ones_rowf")
    nc.vector.memset(c.ones_rowf[:], 1.0)
    c.eps_col = const.tile([128, 1], F32, tag="eps_col")
    nc.vector.memset(c.eps_col[:], EPS)

    # persistent SBUF scratch
    c.rall1 = const.tile([128, NT, RW], F16, tag="rall1")
    c.rall2 = const.tile([128, NT, RW], F16, tag="rall2")
    c.st1 = const.tile([128, NT, 6], F32, tag="st1")
    c.st2 = const.tile([128, NT, 6], F32, tag="st2")
    c.inv1 = const.tile([128, NT], F32, tag="inv1")
    c.inv2 = const.tile([128, NT], F32, tag="inv2")
    return c


def _phase_lin_logits(nc, c, spp, outs):
    sp = c.sp
    # lin1 / lin2: out[b, n] = sum_d img[b,d] W[n,d] + lbias[n]
    for (imt, wT, nch, lbr, oname) in (
            (c.imgT, c.w1iT, C1D, c.lb1r, "lin1_o"),
            (c.ioutT, c.w2iT, C2D, c.lb2r, "lin2_o")):
        for bh in range(2):
            lp = spp.tile([128, NH], F32, tag="lp", bufs=2)
            for ch in range(nch):
                nc.tensor.matmul(lp[:], imt[:, ch, bh * 128:(bh + 1) * 128],
                                 wT[:, ch, :], start=(ch == 0), stop=False)
            nc.tensor.matmul(lp[:], c.ones_rowf[:], lbr[:],
                             start=False, stop=True)
            lsb = sp.tile([128, NH], F32, tag="lsb")
            nc.scalar.copy(lsb[:], lp[:])
            nc.sync.dma_start(outs[oname][bh * 128:(bh + 1) * 128, :], lsb[:])

    # logits: G[b,n] = sum_d img[b,d] * (text[n,d]*es/||t_n||), then * 1/||img_b||
    sq = sp.tile([128, C1D, B], F16, tag="sq")
    nc.vector.tensor_mul(sq[:], c.imgT[:], c.imgT[:])
    for bh in range(2):
        gp = spp.tile([128, NH], F32, tag="lp", bufs=2)
        for ch in range(C1D):
            nc.tensor.matmul(gp[:], c.imgT[:, ch, bh * 128:(bh + 1) * 128],
                             c.ttsT[:, ch, :], start=(ch == 0),
                             stop=(ch == C1D - 1))
        n2 = spp.tile([128, 1], F32, tag="n2", bufs=2)
        for ch in range(C1D):
            nc.tensor.matmul(n2[:], sq[:, ch, bh * 128:(bh + 1) * 128],
                             c.ones_col[:], start=(ch == 0),
                             stop=(ch == C1D - 1))
        nr = sp.tile([128, 1], F32, tag="nr")
        nc.scalar.sqrt(nr[:], n2[:])
        inv_i = sp.tile([128, 1], F32, tag="invi")
        nc.vector.reciprocal(inv_i[:], nr[:])
        lg = sp.tile([128, NH], F32, tag="lsb")
        nc.scalar.activation(lg[:], gp[:], AF.Copy, scale=inv_i[:])
        nc.sync.dma_start(outs["lgt_o"][bh * 128:(bh + 1) * 128, :], lg[:])


def _phase_hidden(nc, c, pools, wm_in, nch, imt, zb, rall, st, ph):
    wmp, zp = pools
    nbuf = int(os.environ.get("KZB", "4"))
    wbuf = int(os.environ.get("KWB", "6"))
    for t in range(NT):
        wm = wmp.tile([128, nch, TR], F16, tag=f"wm{nch}", bufs=wbuf)
        nc.sync.dma_start(wm[:], wm_in[t])
        zps = zp.tile([128, B], F32, tag="zps", bufs=nbuf)
        for ch in range(nch):
            nc.tensor.matmul(zps[:], wm[:, ch, :], imt[:, ch, :],
                             start=(ch == 0), stop=(ch == nch - 1))
        if ph & 4:
            nc.scalar.activation(rall[:, t, 0:B], zps[:], AF.Relu,
                                 bias=zb[:, t:t + 1])
        if ph & 8:
            nc.vector.bn_stats(st[:, t, :], rall[:, t, 0:B])


def _phase_bn_tail(nc, c, st, rall, inv_all):
    # merge even/odd stats (each over 128 of the 256 batch):
    #   mean = (me+mo)/2 ; 256*var = (M2e+M2o) + 64*(me-mo)^2
    sp = c.sp
    me, mo = st[:, :, 1], st[:, :, 4]
    M2e, M2o = st[:, :, 2], st[:, :, 5]
    msum = sp.tile([128, NT], F32, tag="msum")
    nc.vector.tensor_add(msum[:], me, mo)
    dd = sp.tile([128, NT], F32, tag="dd")
    nc.vector.tensor_sub(dd[:], me, mo)
    dd2 = sp.tile([128, NT], F32, tag="dd2")
    nc.vector.tensor_mul(dd2[:], dd[:], dd[:])
    m2s = sp.tile([128, NT], F32, tag="m2s")
    nc.vector.tensor_add(m2s[:], M2e, M2o)
    vv = sp.tile([128, NT], F32, tag="vv")
    nc.vector.scalar_tensor_tensor(vv[:], dd2[:], 64.0, m2s[:],
                                   ALU.mult, ALU.add)
    sv = sp.tile([128, NT], F32, tag="sv")
    nc.scalar.activation(sv[:], vv[:], AF.Sqrt, bias=c.eps_col[:],
                         scale=1.0 / 256.0)
    nc.vector.reciprocal(inv_all[:], sv[:])
    # mean column into r (col B), halved sum
    nc.scalar.activation(rall[:, :, B], msum[:], AF.Copy, scale=0.5)


def _phase_project(nc, c, app, ppp, rall, wg, inv_all, cstr, out_o):
    sp = c.sp
    pp0 = ppp.tile([128, NH], F32, tag="pp0")
    pp1 = ppp.tile([128, NH], F32, tag="pp1")
    ppm = ppp.tile([1, NH], F32, tag="ppm")
    for t in range(NT):
        At = app.tile([128, NH], F16, tag="At", bufs=4)
        nc.vector.tensor_scalar_mul(At[:], wg[:, t, :], inv_all[:, t:t + 1])
        nc.tensor.matmul(pp0[:], rall[:, t, 0:128], At[:],
                         start=(t == 0), stop=False)
        nc.tensor.matmul(pp1[:], rall[:, t, 128:256], At[:],
                         start=(t == 0), stop=False)
        nc.tensor.matmul(ppm[:], rall[:, t, 256:257], At[:],
                         start=(t == 0), stop=(t == NT - 1))
    crow = sp.tile([1, NH], F32, tag="crow")
    nc.vector.tensor_sub(crow[:], cstr[:], ppm[:])
    for bh, pp in ((0, pp0), (1, pp1)):
        nc.tensor.matmul(pp[:], c.ones_rowf[:], crow[:], start=False, stop=True)
        csb = sp.tile([128, NH], F32, tag="lsb")
        nc.scalar.copy(csb[:], pp[:])
        nc.sync.dma_start(out_o[bh * 128:(bh + 1) * 128, :], csb[:])


def _emit_body(nc, tc, ctx, ins, outs):
    # phase bits: 1 lin/logits, 2 hidden matmuls, 4 relu, 8 bn_stats,
    # 16 bn tail, 32 projection. Full kernel = 63.
    PH = int(os.environ.get("KPH", "63"))
    c = _load_persistents(nc, tc, ctx, ins)
    if PH & 1:
        with tc.tile_pool(name="spp", bufs=2, space="PSUM") as spp:
            _phase_lin_logits(nc, c, spp, outs)
    if PH & 2:
        with tc.tile_pool(name="wmp", bufs=12) as wmp, \
             tc.tile_pool(name="zp", bufs=4, space="PSUM") as zp, \
             tc.tile_pool(name="app", bufs=8) as app, \
             tc.tile_pool(name="ppp", bufs=1, space="PSUM") as ppp:
            pools = (wmp, zp)
            _phase_hidden(nc, c, pools, ins["wm1"], C1D, c.imgT, c.zb1,
                          c.rall1, c.st1, PH)
            if PH & 16:
                _phase_bn_tail(nc, c, c.st1, c.rall1, c.inv1)
            _phase_hidden(nc, c, pools, ins["wm2"], C2D, c.ioutT, c.zb2,
                          c.rall2, c.st2, PH)
            if PH & 16:
                _phase_bn_tail(nc, c, c.st2, c.rall2, c.inv2)
            if PH & 32:
                _phase_project(nc, c, app, ppp, c.rall1, c.wg1, c.inv1,
                               c.cst1r, outs["cls1_o"])
                _phase_project(nc, c, app, ppp, c.rall2, c.wg2, c.inv2,
                               c.cst2r, outs["cls2_o"])


def _build(loop_k=1):
    nc = bacc.Bacc("TRN2", target_bir_lowering=False, debug=False,
                   num_devices=NC)
    mk = nc.dram_tensor

    def inp(name, shape, dt):
        return mk(name, shape, dt, kind="ExternalInput").ap()

    ins = {
        "imgT": inp("imgT", [128, C1D * B], F16),
        "ioutT": inp("ioutT", [128, C2D * B], F16),
        "w1iT": inp("w1iT", [128, C1D * NH], F16),
        "w2iT": inp("w2iT", [128, C2D * NH], F16),
        "ttsT": inp("ttsT", [128, C1D * NH], F16),
        "lb1r": inp("lb1r", [1, NH], F32),
        "lb2r": inp("lb2r", [1, NH], F32),
        "cst1r": inp("cst1r", [1, NH], F32),
        "cst2r": inp("cst2r", [1, NH], F32),
        "zb1": inp("zb1", [128, NT], F32),
        "zb2": inp("zb2", [128, NT], F32),
        "wg1": inp("wg1", [128, NT * NH], F16),
        "wg2": inp("wg2", [128, NT * NH], F16),
        "wm1": inp("wm1", [NT, 128, C1D * TR], F16),
        "wm2": inp("wm2", [NT, 128, C2D * TR], F16),
    }
    outs = {
        k: mk(k, [B, NH], F32, kind="ExternalOutput").ap()
        for k in ("lin1_o", "lin2_o", "cls1_o", "cls2_o", "lgt_o")
    }

    with tile.TileContext(nc) as tc:
        with ExitStack() as ctx:
            if loop_k > 1:
                with tc.For_i(0, loop_k, 1):
                    _emit_body(nc, tc, ctx, ins, outs)
            else:
                _emit_body(nc, tc, ctx, ins, outs)
    nc.compile()
    return nc


def _pack_T(x, nch, dtype=np.float16):
    # x: [rows, d] -> [128, nch*rows]; el [p, ch*rows + r] = x[r, ch*128+p]
    rows = x.shape[0]
    return np.ascontiguousarray(
        x.T.reshape(nch, 128, rows).transpose(1, 0, 2).reshape(128, nch * rows)
    ).astype(dtype)


def _pack_wm(w, nch):
    # w: [ROWS, nch*128] -> [NT, 128, nch*TR]; el [t,p,ch*TR+r] = w[TR*t+r, 128*ch+p]
    wp = np.zeros((RP, nch * 128), np.float32)
    wp[:ROWS] = w
    return np.ascontiguousarray(
        wp.reshape(NT, TR, nch, 128).transpose(0, 3, 2, 1).reshape(NT, 128, nch * TR)
    ).astype(np.float16)


def _pack_cols(v):
    # v: [ROWS] -> [128, NT]; col t = v[t*TR:(t+1)*TR] (padded)
    vp = np.zeros((RP,), np.float32)
    vp[:ROWS] = v
    return np.ascontiguousarray(vp.reshape(NT, TR).T)


def _pack_wg(w2g):
    # w2g: [ROWS] -> [128, NT*NH] masked by head: el [p, t*NH+h] = w2g[t*128+p]
    # if (t*128+p)//H == h else 0
    arr = np.zeros((RP, NH), np.float32)
    r = np.arange(ROWS)
    arr[r, r // H] = w2g
    return np.ascontiguousarray(
        arr.reshape(NT, TR, NH).transpose(1, 0, 2).reshape(128, NT * NH)
    ).astype(np.float16)


def host_prep(inputs):
    f32 = np.float32
    g = {k: np.asarray(v, f32) for k, v in inputs.items()}
    image_embed, text_embed = g["image_embed"], g["text_embed"]
    image_out, text_out = g["image_out"], g["text_out"]

    imgT = _pack_T(image_embed, C1D)
    ioutT = _pack_T(image_out, C2D)
    es = np.exp(g["logit_scale"].astype(np.float64)).astype(f32)

    in_maps = []
    for cc in range(NC):
        S = slice(cc * NH, (cc + 1) * NH)
        # B-independent per-row hidden bias: text part of C*_W1 dotted with
        # the per-head text vector, plus C*_b1
        zt1 = np.einsum("nhd,nd->nh", g["C1_W1"][S][:, :, DE:], text_embed[S],
                        optimize=True) + g["C1_b1"][S]
        zt2 = np.einsum("nhd,nd->nh", g["C2_W1"][S][:, :, DV:], text_out[S],
                        optimize=True) + g["C2_b1"][S]

        w2g1 = (g["C1_W2"][S] * g["C1_gamma"][S]).reshape(ROWS)
        w2g2 = (g["C2_W2"][S] * g["C2_gamma"][S]).reshape(ROWS)
        cst1 = g["C1_b2"][S] + (g["C1_W2"][S] * g["C1_beta"][S]).sum(1)
        cst2 = g["C2_b2"][S] + (g["C2_W2"][S] * g["C2_beta"][S]).sum(1)
        lb1 = g["b1"][S] + (text_embed[S] * g["W1"][S, DE:]).sum(1)
        lb2 = g["b2"][S] + (text_out[S] * g["W2"][S, DV:]).sum(1)
        tsc = es / np.linalg.norm(text_embed[S], axis=1)

        in_maps.append({
            "imgT": imgT, "ioutT": ioutT,
            "w1iT": _pack_T(g["W1"][S, :DE], C1D),
            "w2iT": _pack_T(g["W2"][S, :DV], C2D),
            "ttsT": _pack_T(text_embed[S] * tsc[:, None], C1D),
            "lb1r": np.ascontiguousarray(lb1[None, :]),
            "lb2r": np.ascontiguousarray(lb2[None, :]),
            "cst1r": np.ascontiguousarray(cst1[None, :]),
            "cst2r": np.ascontiguousarray(cst2[None, :]),
            "zb1": _pack_cols(zt1.reshape(ROWS)),
            "zb2": _pack_cols(zt2.reshape(ROWS)),
            "wg1": _pack_wg(w2g1),
            "wg2": _pack_wg(w2g2),
            "wm1": _pack_wm(g["C1_W1"][S][:, :, :DE].reshape(ROWS, DE), C1D),
            "wm2": _pack_wm(g["C2_W1"][S][:, :, :DV].reshape(ROWS, DV), C2D),
        })
    return in_maps


_cache = {}


def _get_nc(loop_k=1):
    if loop_k not in _cache:
        _cache[loop_k] = _build(loop_k)
    return _cache[loop_k]


def run(inputs, loop_k=1):
    nc = _get_nc(loop_k)
    in_maps = host_prep(inputs)
    res = run_bass_kernel_spmd(nc, in_maps, core_ids=list(range(NC)))
    names = ("lin1_o", "lin2_o", "cls1_o", "cls2_o", "lgt_o")
    return tuple(
        np.ascontiguousarray(
            np.concatenate([res.results[c][nm] for c in range(NC)], axis=1))
        for nm in names
    )


def kernel(**inputs):
    return run(inputs, loop_k=1)
